# revision 30
# baseline (speedup 1.0000x reference)
"""Trainium2 Bass kernel for nn_Decoder (teacher-forced LSTM decoder w/ attention).

Sharding: data-parallel over batch N=256 across 8 NeuronCores (32 batch/core).
Within a core everything is laid out feature-major ("transposed"): states are
[feat_partitions, batch_free], weights are stationary lhsT tiles, the 300-step
recurrence runs in a For_i loop.

Key structures per core (batch b = 4*g + j; g = group 0..7, j = col-group 0..3):
  - gates matmuls: out[gate_chunk(128p), batch(32f)] accumulated in PSUM, with
    the embedding contribution injected via an identity matmul from a
    precomputed XG table (XG = onehot @ (emb @ Wx^T + b1), computed on-device).
  - attention energy: per-batch matvec as 4 concurrent column-tiled matmuls
    (tile_position=(0,32j)), 4 batches -> one PSUM bank (rows {0,32,64,96}).
  - softmax runs on the full 128-partition bank (garbage rows cost nothing).
  - attn^T / ctx^T obtained via constant selection-matrix matmuls (PSUM
    accumulation across groups) instead of transposes.
  - batches are sorted by decreasing valid length (lens//8) so energy/ctx
    matmuls only process ceil-padded valid keys; softmax masking uses a
    precomputed per-batch mask. The padded slot lengths are shared across all
    8 cores (global sort, snake deal) so one SPMD program serves all cores.
"""
import numpy as np
import ml_dtypes

import concourse.bass as bass
import concourse.bacc as bacc
import concourse.tile as tile
from concourse import mybir
from concourse import bass_utils
from concourse._compat import with_exitstack
from contextlib import ExitStack

BF16 = mybir.dt.bfloat16
F32 = mybir.dt.float32
bf16 = ml_dtypes.bfloat16

V, H, KS, VS = 35, 512, 256, 256
NB, T, MAXLEN = 256, 512, 300
NCORES = 8
B = 32            # batch per core
NG = 8            # groups of 4 per core
U = 4             # steps per For_i iteration
NITER = MAXLEN // U


# ----------------------------------------------------------------------------
# host-side planning
# ----------------------------------------------------------------------------
class Plan:
    def __init__(self, lens8):
        lens8 = np.clip(np.asarray(lens8, dtype=np.int64), 1, T - 1)
        order = np.argsort(-lens8, kind="stable")
        # slot s of every core gets one global group of 4; core c gets group
        # order[(s*8+c)*4 : +4]. Padded slot length = longest in slot.
        self.perm = np.zeros((NCORES, B), dtype=np.int64)
        self.Lhat = np.zeros(NG, dtype=np.int64)
        for s in range(NG):
            for c in range(NCORES):
                g = order[(s * NCORES + c) * 4:(s * NCORES + c) * 4 + 4]
                self.perm[c, 4 * s:4 * s + 4] = g
        for s in range(NG):
            self.Lhat[s] = int(lens8[self.perm[:, 4 * s:4 * s + 4]].max())
        self.Tc = np.maximum(1, np.ceil(self.Lhat / 128).astype(np.int64))
        self.lens8 = lens8

        # consts column map (all bf16, [128, CC])
        off = 0
        def take(n):
            nonlocal off
            o = off
            off += int(n)
            return o
        self.ident_o = take(128)
        self.m1_o = take(16 * 128)
        self.sel_o = take(NG * B)
        self.wc_o = take(2 * 16 * 128)
        self.whh1_o = take(4 * 16 * 128)
        self.wih2_o = take(4 * 8 * 128)
        self.whh2_o = take(2 * 8 * 128)
        self.wout_o = take(4 * V)
        self.b2_o = take(8 * 128)
        self.ones_o = take(B)
        self.kt_o = []
        for b in range(B):
            s = b // 4
            self.kt_o.append(take(2 * self.Lhat[s]))
        # vt chunks carry VS value columns + 1 ones-column (gives the softmax
        # denominator S as ctx-matmul output column VS)
        self.vt_o = []
        for b in range(B):
            s = b // 4
            self.vt_o.append(take(self.Tc[s] * (VS + 1)))
        self.cc = off


def build_onehot(plan, core, text):
    text = np.asarray(text, np.int64)
    oh = np.zeros((128, MAXLEN, B), dtype=np.float32)
    tok = text[plan.perm[core], :MAXLEN]
    for b in range(B):
        oh[tok[b], np.arange(MAXLEN), b] = 1.0
    return oh.reshape(128, MAXLEN * B).astype(bf16)


def build_consts(plan, core, inp):
    """Build the packed [128, cc] bf16 consts array for one core."""
    cc = plan.cc
    A = np.zeros((128, cc), dtype=np.float32)
    perm = plan.perm[core]

    A[:, plan.ident_o:plan.ident_o + 128] = np.eye(128, dtype=np.float32)

    emb = np.asarray(inp["emb"], np.float32)
    W_ih1 = np.asarray(inp["W_ih1"], np.float32)
    W_hh1 = np.asarray(inp["W_hh1"], np.float32)
    W_ih2 = np.asarray(inp["W_ih2"], np.float32)
    W_hh2 = np.asarray(inp["W_hh2"], np.float32)
    W_out = np.asarray(inp["W_out"], np.float32)
    b1 = np.asarray(inp["b_ih1"], np.float32) + np.asarray(inp["b_hh1"], np.float32)
    b2 = np.asarray(inp["b_ih2"], np.float32) + np.asarray(inp["b_hh2"], np.float32)
    text = np.asarray(inp["text"], np.int64)
    enc_key = np.asarray(inp["enc_key"], np.float32)
    enc_values = np.asarray(inp["enc_values"], np.float32)

    # M1[v, 2048] = emb @ Wx^T + b1  (bias folded; onehot rows sum to 1)
    M1 = emb @ W_ih1[:, :H].T + b1[None, :]
    A[0:V, plan.m1_o:plan.m1_o + 16 * 128] = M1

    # Sel_g[32j, 4g+j] = 1
    sel = A[:, plan.sel_o:plan.sel_o + NG * B].reshape(128, NG, B)
    for g in range(NG):
        for j in range(4):
            sel[32 * j, g, 4 * g + j] = 1.0



    # weights, transposed feature-major: [p, kc, mj]
    wc = A[:, plan.wc_o:plan.wc_o + 2 * 16 * 128].reshape(128, 2, 2048)
    for kc in range(2):
        wc[:, kc, :] = W_ih1[:, H + kc * 128:H + (kc + 1) * 128].T
    whh1 = A[:, plan.whh1_o:plan.whh1_o + 4 * 2048].reshape(128, 4, 2048)
    for kc in range(4):
        whh1[:, kc, :] = W_hh1[:, kc * 128:(kc + 1) * 128].T
    wih2 = A[:, plan.wih2_o:plan.wih2_o + 4 * 1024].reshape(128, 4, 1024)
    for kc in range(4):
        wih2[:, kc, :] = W_ih2[:, kc * 128:(kc + 1) * 128].T
    whh2 = A[:, plan.whh2_o:plan.whh2_o + 2 * 1024].reshape(128, 2, 1024)
    for kc in range(2):
        whh2[:, kc, :] = W_hh2[:, kc * 128:(kc + 1) * 128].T
    wout = A[:, plan.wout_o:plan.wout_o + 4 * V].reshape(128, 4, V)
    for kc in range(4):
        wout[:, kc, :] = W_out[:, kc * 128:(kc + 1) * 128].T

    A[0, plan.b2_o:plan.b2_o + 8 * 128] = b2
    A[0, plan.ones_o:plan.ones_o + B] = 1.0

    # Masking is baked into the data: kt/vt columns (and the vt ones-column)
    # are zero for t >= L_n, so masked steps get energy 0 -> weight exp(0)=1,
    # which contributes nothing to ctx' (zero V rows) nor to S (zero ones).
    for b in range(B):
        s = b // 4
        L = int(plan.Lhat[s])
        Ln = int(plan.lens8[perm[b]])
        n = perm[b]
        kt = A[:, plan.kt_o[b]:plan.kt_o[b] + 2 * L].reshape(128, 2, L)
        for kc in range(2):
            kt[:, kc, :Ln] = enc_key[n, :Ln, kc * 128:(kc + 1) * 128].T
        Tc = int(plan.Tc[s])
        vt = A[:, plan.vt_o[b]:plan.vt_o[b] + Tc * (VS + 1)].reshape(128, Tc, VS + 1)
        for tc in range(Tc):
            t0 = tc * 128
            t1 = min(t0 + 128, Ln)
            if t1 > t0:
                vt[0:t1 - t0, tc, 0:VS] = enc_values[n, t0:t1, :]
                vt[0:t1 - t0, tc, VS] = 1.0
    return A.astype(bf16)


# ----------------------------------------------------------------------------
# program builder
# ----------------------------------------------------------------------------
@with_exitstack
def decoder_kernel(ctx: ExitStack, tc_: tile.TileContext, plan: Plan,
                   consts_h, onehot_h, xg_h, preds_h, b2_nonzero: bool,
                   niter: int = NITER, dbg_h=None, lvl: int = 0):
    nc = tc_.nc
    cc = plan.cc

    sb = ctx.enter_context(tc_.tile_pool(name="sb", bufs=1))
    pps = ctx.enter_context(tc_.tile_pool(name="pps", bufs=1, space="PSUM"))

    C = sb.tile([128, cc], BF16)
    nc.sync.dma_start(out=C, in_=consts_h[:, :])

    ident = C[:, plan.ident_o:plan.ident_o + 128]
    selv = C[:, plan.sel_o:plan.sel_o + NG * B].rearrange("p (g b) -> p g b", g=NG)
    wc = C[:, plan.wc_o:plan.wc_o + 2 * 2048].rearrange("p (k m) -> p k m", k=2)
    whh1 = C[:, plan.whh1_o:plan.whh1_o + 4 * 2048].rearrange("p (k m) -> p k m", k=4)
    wih2 = C[:, plan.wih2_o:plan.wih2_o + 4 * 1024].rearrange("p (k m) -> p k m", k=4)
    whh2 = C[:, plan.whh2_o:plan.whh2_o + 2 * 1024].rearrange("p (k m) -> p k m", k=2)
    wout = C[:, plan.wout_o:plan.wout_o + 4 * V].rearrange("p (k v) -> p k v", k=4)
    b2row = C[:, plan.b2_o:plan.b2_o + 8 * 128].rearrange("p (m x) -> p m x", m=8)
    ones = C[:, plan.ones_o:plan.ones_o + B]

    # persistent PSUM banks
    G1A = pps.tile([128, 512], F32, tag="g1a")
    G1B = pps.tile([128, 512], F32, tag="g1b")
    EB0 = pps.tile([128, 512], F32, tag="eb0")
    EB1 = pps.tile([128, 512], F32, tag="eb1")
    AT = pps.tile([128, 512], F32, tag="at")      # attnT [128, tc<=4, 32]
    SH = pps.tile([128, 512], F32, tag="sh")      # g2 [0:256] | ctxT [256:320] | outproj [320:352]
    CXB0 = pps.tile([128, 512], F32, tag="cxb0")  # ctx' [0:256] | S [256]
    CXB1 = pps.tile([128, 512], F32, tag="cxb1")
    G1 = [G1A, G1B]
    EB = [EB0, EB1]
    CXB = [CXB0, CXB1]
    g2ps = SH[:, 0:256]
    ctps = SH[:, 256:320].rearrange("p (k b) -> p k b", k=2)
    opps = SH[0:V, 320:352]

    # persistent sbuf state
    h1T = sb.tile([128, 4, B], BF16, tag="h1T")
    c1 = sb.tile([128, 128], F32, tag="c1")
    h2T = sb.tile([128, 2, B], BF16, tag="h2T")
    c2 = sb.tile([128, 64], F32, tag="c2")
    ctxT = sb.tile([128, 2, B], BF16, tag="ctxT")
    attnT = sb.tile([128, 4, B], BF16, tag="attnT")
    exps = sb.tile([128, NG, 512], BF16, tag="exps")
    cxs = sb.tile([128, NG, 256], BF16, tag="cxs")
    ifs1 = sb.tile([128, 256], F32, tag="ifs1")
    gt1 = sb.tile([128, 128], F32, tag="gt1")
    os1 = sb.tile([128, 128], F32, tag="os1")
    tc1 = sb.tile([128, 128], F32, tag="tc1")
    t1a = sb.tile([128, 128], F32, tag="t1a")
    t1b = sb.tile([128, 128], F32, tag="t1b")
    ifs2 = sb.tile([128, 128], F32, tag="ifs2")
    gt2 = sb.tile([128, 64], F32, tag="gt2")
    os2 = sb.tile([128, 64], F32, tag="os2")
    tc2 = sb.tile([128, 64], F32, tag="tc2")
    t2a = sb.tile([128, 64], F32, tag="t2a")
    t2b = sb.tile([128, 64], F32, tag="t2b")
    rsum = sb.tile([128, NG], F32, tag="rsum")

    # prologue: zero states + psum garbage rows + stale-read tiles
    for t in (h1T, h2T, ctxT):
        nc.vector.memset(t, 0.0)
    for t in (c1, c2):
        nc.vector.memset(t, 0.0)
    nc.vector.memset(exps, 0.0)
    nc.vector.memset(EB0, 0.0)
    nc.vector.memset(EB1, 0.0)
    # garbage rows of CXB are never written; 1.0 keeps reciprocal finite
    nc.vector.memset(CXB0, 1.0)
    nc.vector.memset(CXB1, 1.0)

    # ---------------- XG precompute -----------------------------------------
    QSTEP = 12                      # steps per precompute chunk (384 cols)
    QCOL = QSTEP * B
    nsteps = niter * U
    assert nsteps % QSTEP == 0 and QSTEP % U == 0
    with tc_.tile_pool(name="ohpool", bufs=1) as ohp, \
         tc_.tile_pool(name="xgsb", bufs=3) as xgsb:
        oh = ohp.tile([128, MAXLEN * B], BF16)
        nc.sync.dma_start(out=oh, in_=onehot_h[:, :])
        for m in range(16):
            m1t = C[0:V, plan.m1_o + m * 128:plan.m1_o + (m + 1) * 128]
            for q in range(nsteps // QSTEP):
                ps = G1[q % 2][:, 0:QCOL]
                nc.tensor.matmul(ps, m1t, oh[0:V, q * QCOL:(q + 1) * QCOL],
                                 start=True, stop=True)
                xsb = xgsb.tile([128, QCOL], BF16)
                if (m + q) % 2 == 0:
                    nc.vector.tensor_copy(xsb, ps)
                else:
                    nc.scalar.activation(xsb, ps, mybir.ActivationFunctionType.Copy)
                # cols (t_local, b): t = q*QSTEP + tl ; iter = t // U, u = t % U
                src = xsb.rearrange("p (i x) -> p i x", i=QSTEP // U)
                dst = xg_h[q * (QSTEP // U):(q + 1) * (QSTEP // U), :, m, :, :]
                dst = dst.rearrange("i p u b -> p i (u b)")
                nc.sync.dma_start(out=dst, in_=src)
    xgpool = ctx.enter_context(tc_.tile_pool(name="xgpool", bufs=2))
    prpool = ctx.enter_context(tc_.tile_pool(name="prpool", bufs=2))

    # ---------------- main loop --------------------------------------------
    lhat = [int(x) for x in plan.Lhat]
    tcs = [int(x) for x in plan.Tc]
    TCMAX = max(tcs)

    def emit_gates_pre(u, xgv):
        """Return closures for gates1 matmuls that do NOT need ctxT(u-1):
        the XG inject and the W_hh1*h1 part. Woven into the previous
        substep's attention phase as PE filler."""
        g1 = G1[u % 2]
        jobs = [lambda: nc.tensor.matmul(
            g1.rearrange("p (m b) -> p m b", m=16), ident, xgv,
            start=True, stop=False, skip_group_check=True)]
        for m in range(16):
            reg = g1[:, m * 32:(m + 1) * 32]
            for kc in range(4):
                jobs.append(lambda reg=reg, kc=kc, m=m: nc.tensor.matmul(
                    reg, whh1[:, kc, m * 128:(m + 1) * 128],
                    h1T[:, kc, :], start=False, stop=False, skip_group_check=True))
        return jobs

    def emit_step(u, xgv, predv, pre_jobs):
        g1 = G1[u % 2]
        # gates1: ctx part (inject + h1 part already emitted in the weave)
        for m in range(16):
            reg = g1[:, m * 32:(m + 1) * 32]
            for kc in range(2):
                nc.tensor.matmul(reg, wc[:, kc, m * 128:(m + 1) * 128],
                                 ctxT[:, kc, :], start=False, stop=(kc == 1), skip_group_check=True)
        # pointwise 1
        nc.scalar.activation(ifs1, g1[:, 0:256], mybir.ActivationFunctionType.Sigmoid)
        nc.scalar.activation(gt1, g1[:, 256:384], mybir.ActivationFunctionType.Tanh)
        nc.scalar.activation(os1, g1[:, 384:512], mybir.ActivationFunctionType.Sigmoid)
        nc.vector.tensor_mul(t1a, ifs1[:, 128:256], c1)
        nc.vector.tensor_mul(t1b, ifs1[:, 0:128], gt1)
        nc.vector.tensor_add(c1, t1a, t1b)
        nc.scalar.activation(tc1, c1, mybir.ActivationFunctionType.Tanh)
        nc.vector.tensor_mul(h1T.rearrange("p a b -> p (a b)"), os1, tc1)
        # gates2
        for m in range(8):
            reg = g2ps[:, m * 32:(m + 1) * 32]
            for kc in range(4):
                nc.tensor.matmul(reg, wih2[:, kc, m * 128:(m + 1) * 128],
                                 h1T[:, kc, :], start=(kc == 0), stop=False, skip_group_check=True)
            for kc in range(2):
                last = (kc == 1) and not b2_nonzero
                nc.tensor.matmul(reg, whh2[:, kc, m * 128:(m + 1) * 128],
                                 h2T[:, kc, :], start=False, stop=last, skip_group_check=True)
            if b2_nonzero:
                nc.tensor.matmul(reg, b2row[0:1, m, :], ones[0:1, :],
                                 start=False, stop=True, skip_group_check=True)
        # pointwise 2
        nc.scalar.activation(ifs2, g2ps[:, 0:128], mybir.ActivationFunctionType.Sigmoid)
        nc.scalar.activation(gt2, g2ps[:, 128:192], mybir.ActivationFunctionType.Tanh)
        nc.scalar.activation(os2, g2ps[:, 192:256], mybir.ActivationFunctionType.Sigmoid)
        nc.vector.tensor_mul(t2a, ifs2[:, 64:128], c2)
        nc.vector.tensor_mul(t2b, ifs2[:, 0:64], gt2)
        nc.vector.tensor_add(c2, t2a, t2b)
        nc.scalar.activation(tc2, c2, mybir.ActivationFunctionType.Tanh)
        nc.vector.tensor_mul(h2T.rearrange("p a b -> p (a b)"), os2, tc2)

        # attention, software-pipelined: energy(g) -> [filler matmuls from the
        # next substep's gates1] -> sel(g-1), so the PE never waits on exp(g).
        # (no max shift: energies are small; mask is baked into kt/vt zeros)
        def emit_sel(g):
            for tcc in range(tcs[g]):
                nc.tensor.matmul(
                    AT[:, tcc * 32:(tcc + 1) * 32],
                    exps[0:97, g, tcc * 128:(tcc + 1) * 128],
                    selv[0:97, g, :],
                    start=(g == 0 and tcc == 0), stop=(g == NG - 1),
                    skip_group_check=True)

        for g in range(NG):
            L = lhat[g]
            eb = EB[g % 2]
            for j in range(4):
                b = 4 * g + j
                for kc in range(2):
                    nc.tensor.matmul(
                        eb[32 * j:32 * j + 1, 0:L],
                        h2T[:, kc, b:b + 1],
                        C[:, plan.kt_o[b] + kc * L:plan.kt_o[b] + (kc + 1) * L],
                        start=(kc == 0), stop=(kc == 1),
                        tile_position=(0, 32 * j), skip_group_check=True)
            nc.scalar.activation(exps[:, g, 0:L], eb[:, 0:L],
                                 mybir.ActivationFunctionType.Exp)
            for _ in range(13):
                if pre_jobs:
                    pre_jobs.pop(0)()
            if g > 0:
                emit_sel(g - 1)
        while pre_jobs:
            pre_jobs.pop(0)()
        emit_sel(NG - 1)
        nc.vector.tensor_copy(
            attnT.rearrange("p a b -> p (a b)")[:, 0:TCMAX * 32],
            AT[:, 0:TCMAX * 32])
        # ctx' + S (ones-column); normalization folded into the cxs copy
        for g in range(NG):
            cxp = CXB[g % 2][:, 0:VS + 1]
            for j in range(4):
                b = 4 * g + j
                for tcc in range(tcs[g]):
                    o = plan.vt_o[b] + tcc * (VS + 1)
                    nc.tensor.matmul(
                        cxp[32 * j:32 * j + 1, :],
                        attnT[:, tcc, b:b + 1],
                        C[:, o:o + VS + 1],
                        start=(tcc == 0), stop=(tcc == tcs[g] - 1),
                        tile_position=(0, 32 * j), skip_group_check=True)
            nc.vector.reciprocal(rsum[:, g:g + 1], cxp[:, VS:VS + 1])
            nc.vector.tensor_scalar_mul(cxs[:, g, :], cxp[0:128, 0:VS],
                                        rsum[:, g:g + 1])
        # ctxT via Sel accumulation
        for g in range(NG):
            for vc in range(2):
                nc.tensor.matmul(ctps[:, vc, :],
                                 cxs[0:97, g, vc * 128:(vc + 1) * 128],
                                 selv[0:97, g, :],
                                 start=(g == 0 and vc == 0),
                                 stop=(g == NG - 1 and vc == 1),
                                 skip_group_check=True)
        nc.vector.tensor_copy(ctxT.rearrange("p a b -> p (a b)"),
                              ctps.rearrange("p a b -> p (a b)"))
        # output projection
        for kc in range(4):
            rhs = h2T[:, kc, :] if kc < 2 else ctxT[:, kc - 2, :]
            nc.tensor.matmul(opps, wout[:, kc, :], rhs,
                             start=(kc == 0), stop=(kc == 3), skip_group_check=True)
        nc.vector.tensor_copy(predv, opps)

    with tc_.For_i(0, niter, 1, hint_engines=(mybir.EngineType.PE,
                                              mybir.EngineType.DVE,
                                              mybir.EngineType.Activation)) as iv:
        slab = xgpool.tile([128, U * 512], BF16)
        nc.sync.dma_start(out=slab,
                          in_=xg_h[bass.ds(iv, 1)].rearrange("o p m u b -> p (o m u b)"))
        predsb = prpool.tile([V, U, B], F32)
        slab4 = slab.rearrange("p (m u b) -> p m u b", m=16, u=U)
        # substep 0's inject+whh1 are emitted directly; substeps 1..U-1 get
        # theirs woven into the previous substep's attention phase
        for j_ in emit_gates_pre(0, slab4[:, :, 0, :]):
            j_()
        for u in range(U):
            xgv = slab4[:, :, u, :]
            nxt = emit_gates_pre(u + 1, slab4[:, :, u + 1, :]) if u + 1 < U else []
            emit_step(u, xgv, predsb[:, u, :], nxt)
            if dbg_h is not None and u == 0:
                dsb = prpool.tile([128, 1408], BF16, tag="dsb")
                nc.vector.tensor_copy(dsb[:, 0:128], h1T.rearrange("p a b -> p (a b)"))
                nc.vector.tensor_copy(dsb[:, 128:192], h2T.rearrange("p a b -> p (a b)"))
                nc.vector.tensor_copy(dsb[:, 192:256], ctxT.rearrange("p a b -> p (a b)"))
                nc.vector.tensor_copy(dsb[:, 256:384], attnT.rearrange("p a b -> p (a b)"))
                nc.vector.tensor_copy(dsb[:, 384:896], exps[:, 0, :])
                nc.vector.tensor_copy(dsb[:, 896:1152], cxs[:, 0, :])
                nc.vector.tensor_copy(dsb[:, 1152:1408], cxs[:, 1, :])
                nc.sync.dma_start(out=dbg_h[bass.ds(iv, 1)].rearrange("o p x -> p (o x)"), in_=dsb)
        nc.sync.dma_start(out=preds_h[bass.ds(iv, 1)].rearrange("o v u b -> v u (o b)"),
                          in_=predsb)


# ----------------------------------------------------------------------------
# entry point
# ----------------------------------------------------------------------------
_CACHE = {}
LAST_EXEC_NS = None
LAST_TRACE_PATH = None


def _build_program(plan, b2_nonzero, niter=NITER, dbg=False, lvl=0):
    nc = bacc.Bacc("TRN2", debug=False)
    consts_h = nc.dram_tensor("consts", [128, plan.cc], BF16, kind="ExternalInput")
    onehot_h = nc.dram_tensor("onehot", [128, MAXLEN * B], BF16, kind="ExternalInput")
    xg_h = nc.dram_tensor("xg", [niter, 128, 16, U, B], BF16, kind="Internal")
    preds_h = nc.dram_tensor("preds", [niter, V, U, B], F32, kind="ExternalOutput")
    dbg_h = nc.dram_tensor("dbg", [niter, 128, 1408], BF16, kind="ExternalOutput") if dbg else None
    with tile.TileContext(nc) as tc_:
        decoder_kernel(tc_, plan, consts_h, onehot_h, xg_h[:, :, :, :, :],
                       preds_h, b2_nonzero, niter=niter, dbg_h=dbg_h, lvl=lvl)
    nc.compile()
    return nc


def kernel(**inp):
    lens = np.asarray(inp["lens"], np.int64)
    lens8 = lens // 8
    plan = Plan(lens8)
    b2 = np.asarray(inp["b_ih2"], np.float32) + np.asarray(inp["b_hh2"], np.float32)
    b2_nonzero = bool(np.any(b2 != 0.0))

    key = (tuple(plan.Lhat), b2_nonzero)
    if key not in _CACHE:
        _CACHE[key] = _build_program(plan, b2_nonzero)
    nc = _CACHE[key]

    in_maps = []
    for c in range(NCORES):
        A = build_consts(plan, c, inp)
        OH = build_onehot(plan, c, inp["text"])
        in_maps.append({"consts": A, "onehot": OH})
    res = bass_utils.run_bass_kernel_spmd(nc, in_maps, core_ids=list(range(NCORES)))
    global LAST_EXEC_NS
    if getattr(res, "exec_time_ns", None):
        LAST_EXEC_NS = res.exec_time_ns
        it = getattr(res, "instructions_and_trace", None)
        if it:
            LAST_TRACE_PATH = it[1]
            print(f"[kernel] exec_time_ns={res.exec_time_ns} trace={it[1]}")

    b_out = np.asarray(inp["b_out"], np.float32)
    out = np.zeros((NB, MAXLEN, V), dtype=np.float32)
    for c in range(NCORES):
        p = res.results[c]["preds"]            # [NITER, V, U, B]
        p = np.transpose(p, (3, 0, 2, 1)).reshape(B, MAXLEN, V)
        out[plan.perm[c]] = p
    out += b_out[None, None, :]
    return out


if __name__ == "__main__":
    # quick self-run with random data
    rng = np.random.default_rng(0)
    inp = {
        "enc_key": rng.standard_normal((NB, T, KS), dtype=np.float32),
        "enc_values": rng.standard_normal((NB, T, VS), dtype=np.float32),
        "text": rng.integers(0, V, (NB, MAXLEN + 1)),
        "lens": rng.integers(8, T * 8, (NB,)),
        "teach": 1,
        "emb": rng.standard_normal((V, H), dtype=np.float32) * 0.05,
        "W_ih1": rng.standard_normal((4 * H, H + VS), dtype=np.float32) * 0.05,
        "W_hh1": rng.standard_normal((4 * H, H), dtype=np.float32) * 0.05,
        "b_ih1": np.zeros(4 * H, np.float32),
        "b_hh1": np.zeros(4 * H, np.float32),
        "W_ih2": rng.standard_normal((4 * KS, H), dtype=np.float32) * 0.05,
        "W_hh2": rng.standard_normal((4 * KS, KS), dtype=np.float32) * 0.05,
        "b_ih2": np.zeros(4 * KS, np.float32),
        "b_hh2": np.zeros(4 * KS, np.float32),
        "W_out": rng.standard_normal((V, KS + VS), dtype=np.float32) * 0.05,
        "b_out": np.zeros(V, np.float32),
    }
    out = kernel(**inp)
    print("out", out.shape, out.dtype, np.abs(out).max())

